# revision 42
# baseline (speedup 1.0000x reference)
"""Hawkes process log-likelihood on 8 Trainium2 NeuronCores (~25.5us HW).

Factorization: the pairwise kernel exponent
    E_ij = log(c) - beta*(t_i - t_j) - ||s_i - s_j||^2 / (2 sigma^2)
with c = alpha*beta/(2 pi sigma^2) splits (with per-batch centered coords) as
    E_ij = (a_i + b_j) + (x_i*x_j + y_i*y_j)/sigma^2
    a_i  = log(c) - beta*t_i - (x_i^2+y_i^2)/(2 sigma^2)
    b_j  =          beta*t_j - (x_j^2+y_j^2)/(2 sigma^2)
so a [128 x <=512] tile of E is ONE K=4 fp32r matmul (lhsT rows
[x_i, y_i, 1, a_i]; rhs rows [x_j/s2, y_j/s2, b_j, 1]); fp32r runs at bf16
rate (1 cyc/row) vs 4 for fp32. ScalarE then fuses exp + row-sum in a single
activation per row-tile (accum_out).

Causality: each 128-row tile i needs history columns [0, 128*(i+1)), further
narrowed by the temporal-decay cutoff (beta*dt > logc+95 underflows to 0 in
f32). The host packs the surviving span as chunks in reverse order, so chunk
0 always ends exactly at the diagonal block: the strict-lower-triangular
mask is a single fixed [128,128] -1e30 tile (generated on-chip with
affine_select) VectorE-added at chunk 0's last 128 psum cols, and
padding/pre-window columns are poisoned host-side with b=-1e30 (exp -> 0).
All spans are maxed over cores so the SPMD program is identical.

Load balance: 64 row-tiles processed as complementary pairs (i, 15-i),
grouped so every core's slot k sees i in {2k, 2k+1} -> ~identical spans and
work everywhere. Latency hiding: slots run in ascending-span order; the
first slot's lhsT+rhs ride one small head DMA; rhs transfers are spread
over the sync/gpsimd/scalar DGE queues in consumption order; a dummy exp
pulls the ACT table load into the preamble; a dummy fp32 matmul fills the
PE-idle DMA window so the HAM clock gate flips to 2.4 GHz early.

Per-core output is the row-sum matrix lam [128,8]; the host adds mu[cls],
takes log, and reduces in float64.
"""

import math
from contextlib import ExitStack

import numpy as np

import concourse.bass as bass
import concourse.tile as tile
from concourse import bacc, mybir
from concourse.bass_utils import run_bass_kernel_spmd

# Problem constants (from the reference nn.Module)
T0, T1 = 0.0, 365.0
KM_PER_LON = 111.32 * 0.772
KM_PER_LAT = 110.574
EPS = 1e-5
NEG_BIG = -1e30

B, L = 4, 2048
NCORES = 8
NRT = 16          # row tiles per batch (L/128)
CHUNK = 512

LAST_EXEC_NS = None
_PROFILE = False
_TRACE_KW = {}


def _build_nc(CW):
    f32 = mybir.dt.float32
    f32r = mybir.dt.float32r
    nc = bacc.Bacc(None, target_bir_lowering=False)

    spans = [sum(cw) for cw in CW]
    pscols = -(-max(spans) // CHUNK) * CHUNK
    psbufs = max(2, 8 // (pscols // CHUNK))

    lhsT_d = nc.dram_tensor(
        "lhsT", [4, 8 * 128 + min(spans)], f32r, kind="ExternalInput"
    )
    rhs_d = nc.dram_tensor("rhs", [4, 8 * L], f32r, kind="ExternalInput")
    out_d = nc.dram_tensor("lam", [128, 8], f32, kind="ExternalOutput")

    with tile.TileContext(nc) as tc, ExitStack() as ctx:
        singles = ctx.enter_context(tc.tile_pool(name="singles", bufs=1))
        sc_pool = ctx.enter_context(tc.tile_pool(name="scratch", bufs=2))
        ps_pool = ctx.enter_context(
            tc.tile_pool(name="psum", bufs=psbufs, space="PSUM")
        )

        # zeros for the PE warm-up matmul -- first thing gpsimd does
        zmm_t = singles.tile([4, 384], f32)
        nc.gpsimd.memset(zmm_t[:], 0.0)

        # strict-lower-triangular causal mask: tri[r, c] = 0 if c < r else
        # -1e30. Generated before gpsimd's DMA issues: it gates the first
        # exp via the diagonal-block mask add.
        tri_t = singles.tile([128, 128], f32)
        nc.gpsimd.memset(tri_t[:], 0.0)
        nc.gpsimd.affine_select(
            out=tri_t[:],
            in_=tri_t[:],
            compare_op=mybir.AluOpType.is_ge,
            fill=NEG_BIG,
            base=-1,
            pattern=[[-1, 128]],
            channel_multiplier=1,
        )

        # Process slots in ascending-span order: small slots ramp the
        # pipeline while the bigger transfers are still in flight.
        order = sorted(range(8), key=lambda s: spans[s])

        # DMA order = dependency order of the matmul->mask->exp chains.
        # Each engine's DGE runs one queue (issue ~0.7us, ~1.3us first-byte,
        # transfers serial per queue), so slots are spread over
        # sync/scalar/gpsimd by the time their data is consumed, and the
        # whole first chain (lhsT + a-row + first rhs) rides in ONE transfer.
        rhs_ts = [
            singles.tile([4, spans[s]], f32r, tag=f"rhs{s}", name=f"rhs{s}")
            for s in range(8)
        ]
        # first chain rides in a small head DMA: slot order[0]'s lhsT column
        # block sits at the head's front, its rhs (incl ones row) at the tail
        lhsT_t = singles.tile([4, 8 * 128 + spans[order[0]]], f32r)
        nc.sync.dma_start(
            lhsT_t[:, 8 * 128 :], lhsT_d[:, 8 * 128 :]
        )
        nc.sync.dma_start(
            lhsT_t[:, 128 * order[0] : 128 * (order[0] + 1)],
            lhsT_d[:, 128 * order[0] : 128 * (order[0] + 1)],
        )
        for s in (order[1], order[2]):
            nc.gpsimd.dma_start(
                rhs_ts[s][:], rhs_d[:, L * s : L * s + spans[s]]
            )
        for s in (order[6], order[7]):
            nc.scalar.dma_start(
                rhs_ts[s][:], rhs_d[:, L * s : L * s + spans[s]]
            )
        if order[0] > 0:
            nc.sync.dma_start(
                lhsT_t[:, 0 : 128 * order[0]], lhsT_d[:, 0 : 128 * order[0]]
            )
        nc.sync.dma_start(
            lhsT_t[:, 128 * (order[0] + 1) : 8 * 128],
            lhsT_d[:, 128 * (order[0] + 1) : 8 * 128],
        )
        for s in (order[3], order[4], order[5]):
            nc.sync.dma_start(
                rhs_ts[s][:], rhs_d[:, L * s : L * s + spans[s]]
            )

        # dummy exp on a zeroed element: forces the exp ACT_TABLE_LOAD
        # (~1.3us) into the idle preamble window instead of before the
        # first real ACTIVATE on the critical path
        warm_t = singles.tile([1, 1], f32)
        nc.gpsimd.memset(warm_t[:], 0.0)
        nc.scalar.activation(
            warm_t[:], warm_t[:], mybir.ActivationFunctionType.Exp
        )

        # one dummy fp32 matmul (~1.7us at 4 cyc/row) fills the PE-idle
        # DMA-latency window with activity, so the HAM clock gate flips to
        # 2.4 GHz shortly after the real matmuls start; sized to end right
        # as the first slot's data lands
        ps_warm = ps_pool.tile([128, pscols], f32, tag="ps")
        nc.tensor.matmul(
            ps_warm[:, 0:384],
            zmm_t[:, 0:128],
            zmm_t[:, 0:384],
            start=True,
            stop=True,
        )

        # lam columns are indexed by PROCESSING position (not slot id) so
        # the first half can ship mid-kernel; the host maps them back.
        lam_t = singles.tile([128, 8], f32)

        for k, s in enumerate(order):
            cw = CW[s]
            rhs_ap = lhsT_t[:, 8 * 128 :] if s == order[0] else rhs_ts[s][:]
            ps = ps_pool.tile([128, pscols], f32, tag="ps")
            off = 0
            for g, w in enumerate(cw):
                nc.tensor.matmul(
                    ps[:, off : off + w],
                    lhsT_t[:, 128 * s : 128 * (s + 1)],
                    rhs_ap[:, off : off + w],
                    start=True,
                    stop=True,
                )
                off += w
            # causal mask on the diagonal block (last 128 cols of chunk 0)
            nc.vector.tensor_add(
                ps[:, cw[0] - 128 : cw[0]], ps[:, cw[0] - 128 : cw[0]], tri_t[:]
            )
            et = sc_pool.tile([128, pscols], f32)
            nc.scalar.activation(
                et[:, : spans[s]],
                ps[:, : spans[s]],
                mybir.ActivationFunctionType.Exp,
                accum_out=lam_t[:, k : k + 1],
            )
            if k == 3:
                # first half of the output overlaps the remaining slots
                nc.sync.dma_start(out_d[:, 0:4], lam_t[:, 0:4])

        # issued by ScalarE itself right after its last accum readout --
        # no cross-engine semaphore hop on the tail
        nc.scalar.dma_start(out_d[:, 4:8], lam_t[:, 4:8])

    nc.compile()
    return nc


def _pack_inputs(X, mu, alpha, beta, sigma):
    """Host-side f64 prep: per-core input dicts for the SPMD kernel.

    Returns (in_maps, mug_slots, W) where mug_slots[c] is the [128, 8] matrix
    of mu[cls] for the host-side finalize and W[s] is the chunk count of slot
    s (identical across cores; data-driven via the temporal-decay cutoff)."""
    t = X[..., 0].astype(np.float64)
    cls = X[..., 1].astype(np.int32)
    lon = X[..., 2].astype(np.float64)
    lat = X[..., 3].astype(np.float64)
    alpha = float(alpha)
    beta = float(beta)
    sigma = float(sigma)

    sig2 = sigma * sigma
    two_sig2 = 2.0 * sig2
    logc = math.log(alpha * beta / (math.pi * two_sig2))

    # per-batch centering (E is invariant; keeps fp32 magnitudes small)
    xc = lon - lon.mean(axis=1, keepdims=True)
    yc = lat - lat.mean(axis=1, keepdims=True)
    tc_ = t - t.mean(axis=1, keepdims=True)

    q = (xc * xc + yc * yc) / two_sig2
    a = logc - beta * tc_ - q          # [B, L]
    bv = beta * tc_ - q                # [B, L]
    rx = xc / sig2
    ry = yc / sig2
    mug = np.asarray(mu, np.float64)[cls]  # [B, L]

    # complementary row-tile pairs (i, 15-i), grouped so every core's slot s
    # sees nearly the same i (minimizes the max-over-cores span per slot):
    # group k holds i in {2k, 2k+1} -> 8 pairs, one per core.
    core_slots = []
    for c in range(NCORES):
        slots = []
        for k in range(4):
            b, i = c // 2, 2 * k + (c % 2)
            slots += [(b, i), (b, NRT - 1 - i)]
        core_slots.append(slots)

    # Temporal-decay cutoff: a dropped column contributes at most
    # exp(logc - beta*dt) to lam, and lam >= min(mu), so requiring
    # sum(dropped)/lam <= L*exp(logc - beta*dt_cut)/min(mu) <= e^-20 (~2e-9,
    # far below the reference's own f32 ulp) bounds the error of dropping
    # all history older than dt_cut. Falls back to the f32-underflow cut if
    # mu is not positive. Spans must be identical across cores (one SPMD
    # program) -> max over cores per slot, rounded to 128. Chunk 0 (ending
    # at the diagonal) is at most 512 wide; the rest splits into <=512
    # chunks.
    mu_min = float(np.min(np.asarray(mu, np.float64)))
    if beta > 0:
        if mu_min > 0:
            cut = (logc + math.log(L) - math.log(mu_min) + 20.0) / beta
        else:
            cut = (logc + 95.0) / beta
    else:
        cut = np.inf
    spans = [128] * 8
    for c in range(NCORES):
        for s, (b, i) in enumerate(core_slots[c]):
            d = 128 * (i + 1)
            j_min = int(np.searchsorted(t[b], t[b, 128 * i] - cut))
            need = -(-max(d - j_min, 128) // 128) * 128
            spans[s] = max(spans[s], min(need, d))
    CW = []
    for sp in spans:
        cw = [min(CHUNK, sp)]
        rem = sp - cw[0]
        while rem > 0:
            cw.append(min(CHUNK, rem))
            rem -= cw[-1]
        CW.append(cw)

    in_maps = []
    mug_slots = []
    for c in range(NCORES):
        slots = core_slots[c]
        # lhsT rows: [xc, yc, 1, a]; rhs rows: [rx, ry, bv, 1] ->
        # E = xc*rx + yc*ry + bv + a  (bias folded into the matmul).
        # The first-processed slot's rhs rides in lhsT's tail so the first
        # chain is one DMA.
        o0 = min(range(8), key=lambda s: (spans[s], s))
        lhsT = np.zeros((4, 8 * 128 + spans[o0]), np.float32)
        rhs = np.zeros((4, 8 * L), np.float32)
        mugp = np.zeros((128, 8), np.float64)
        for s, (b, i) in enumerate(slots):
            sp = spans[s]
            cw = CW[s]
            rows = slice(128 * i, 128 * (i + 1))
            lhsT[0, 128 * s : 128 * (s + 1)] = xc[b, rows]
            lhsT[1, 128 * s : 128 * (s + 1)] = yc[b, rows]
            lhsT[2, 128 * s : 128 * (s + 1)] = 1.0
            lhsT[3, 128 * s : 128 * (s + 1)] = a[b, rows]
            mugp[:, s] = mug[b, rows]

            # history span [d - sp, d) packed so chunk 0 ends exactly at the
            # diagonal, then earlier cols in <=512 chunks; padding cols (< 0)
            # are poisoned with b = -1e30 -> exp -> 0.
            d = 128 * (i + 1)
            lo = d - sp
            pad = -lo if lo < 0 else 0
            span = np.zeros((4, sp), np.float32)
            span[2, :pad] = NEG_BIG
            span[3, :] = 1.0
            cols = slice(max(lo, 0), d)
            span[0, pad:] = rx[b, cols]
            span[1, pad:] = ry[b, cols]
            span[2, pad:] = bv[b, cols]
            off = 0
            hi = sp
            for w in cw:
                rhs[:, L * s + off : L * s + off + w] = span[:, hi - w : hi]
                off += w
                hi -= w
        lhsT[:, 8 * 128 :] = rhs[:, L * o0 : L * o0 + spans[o0]]
        in_maps.append({"lhsT": lhsT, "rhs": rhs})
        mug_slots.append(mugp)
    return in_maps, mug_slots, CW


def kernel(X, mu, alpha, beta, sigma):
    global LAST_EXEC_NS
    X = np.asarray(X)
    mu64 = np.asarray(mu, np.float64)
    in_maps, mug_slots, W = _pack_inputs(X, mu, alpha, beta, sigma)
    nc = _build_nc(W)

    kwargs = {}
    if _PROFILE:
        kwargs = dict(trace=True, trace_cores=list(range(NCORES)), **_TRACE_KW)
    res = run_bass_kernel_spmd(nc, in_maps, core_ids=list(range(NCORES)), **kwargs)
    LAST_EXEC_NS = res.exec_time_ns

    spans = [sum(cw) for cw in W]
    order = sorted(range(8), key=lambda s: spans[s])
    sumlog = 0.0
    for c in range(NCORES):
        lam = res.results[c]["lam"].astype(np.float64)
        mug_eff = mug_slots[c][:, list(order)]
        sumlog += float(np.log(lam + mug_eff + EPS).sum())
    area = ((-0.30 - -0.42) * KM_PER_LON) * ((39.52 - 39.40) * KM_PER_LAT)
    baserate = float(mu64.sum()) * (T1 - T0) * area * B
    return np.float32(sumlog - baserate)


# revision 43
# speedup vs baseline: 1.0262x; 1.0262x over previous
"""Hawkes process log-likelihood on 8 Trainium2 NeuronCores (~25.5us HW).

Factorization: the pairwise kernel exponent
    E_ij = log(c) - beta*(t_i - t_j) - ||s_i - s_j||^2 / (2 sigma^2)
with c = alpha*beta/(2 pi sigma^2) splits (with per-batch centered coords) as
    E_ij = (a_i + b_j) + (x_i*x_j + y_i*y_j)/sigma^2
    a_i  = log(c) - beta*t_i - (x_i^2+y_i^2)/(2 sigma^2)
    b_j  =          beta*t_j - (x_j^2+y_j^2)/(2 sigma^2)
so a [128 x <=512] tile of E is ONE K=4 fp32r matmul (lhsT rows
[x_i, y_i, 1, a_i]; rhs rows [x_j/s2, y_j/s2, b_j, 1]); fp32r runs at bf16
rate (1 cyc/row) vs 4 for fp32. ScalarE then fuses exp + row-sum in a single
activation per row-tile (accum_out).

Causality: each 128-row tile i needs history columns [0, 128*(i+1)), further
narrowed by the temporal-decay cutoff (beta*dt > logc+95 underflows to 0 in
f32). The host packs the surviving span as chunks in reverse order, so chunk
0 always ends exactly at the diagonal block: the strict-lower-triangular
mask is a single fixed [128,128] -1e30 tile (generated on-chip with
affine_select) VectorE-added at chunk 0's last 128 psum cols, and
padding/pre-window columns are poisoned host-side with b=-1e30 (exp -> 0).
All spans are maxed over cores so the SPMD program is identical.

Load balance: 64 row-tiles processed as complementary pairs (i, 15-i),
grouped so every core's slot k sees i in {2k, 2k+1} -> ~identical spans and
work everywhere. Latency hiding: slots run in ascending-span order; the
first slot's lhsT+rhs ride one small head DMA; rhs transfers are spread
over the sync/gpsimd/scalar DGE queues in consumption order; a dummy exp
pulls the ACT table load into the preamble; a dummy fp32 matmul fills the
PE-idle DMA window so the HAM clock gate flips to 2.4 GHz early.

Per-core output is the row-sum matrix lam [128,8]; the host adds mu[cls],
takes log, and reduces in float64.
"""

import math
from contextlib import ExitStack

import numpy as np

import concourse.bass as bass
import concourse.tile as tile
from concourse import bacc, mybir
from concourse.bass_utils import run_bass_kernel_spmd

# Problem constants (from the reference nn.Module)
T0, T1 = 0.0, 365.0
KM_PER_LON = 111.32 * 0.772
KM_PER_LAT = 110.574
EPS = 1e-5
NEG_BIG = -1e30

B, L = 4, 2048
NCORES = 8
NRT = 16          # row tiles per batch (L/128)
CHUNK = 512

LAST_EXEC_NS = None
_PROFILE = False
_TRACE_KW = {}


def _build_nc(CW):
    f32 = mybir.dt.float32
    f32r = mybir.dt.float32r
    nc = bacc.Bacc(None, target_bir_lowering=False)

    spans = [sum(cw) for cw in CW]
    pscols = -(-max(spans) // CHUNK) * CHUNK
    psbufs = max(2, 8 // (pscols // CHUNK))

    lhsT_d = nc.dram_tensor(
        "lhsT", [4, 8 * 128 + min(spans)], f32r, kind="ExternalInput"
    )
    rhs_d = nc.dram_tensor("rhs", [4, 8 * L], f32r, kind="ExternalInput")
    out_d = nc.dram_tensor("lam", [128, 8], f32, kind="ExternalOutput")

    with tile.TileContext(nc) as tc, ExitStack() as ctx:
        singles = ctx.enter_context(tc.tile_pool(name="singles", bufs=1))
        sc_pool = ctx.enter_context(tc.tile_pool(name="scratch", bufs=2))
        ps_pool = ctx.enter_context(
            tc.tile_pool(name="psum", bufs=psbufs, space="PSUM")
        )

        # zeros for the PE warm-up matmul -- first thing gpsimd does
        zmm_t = singles.tile([4, 384], f32)
        nc.gpsimd.memset(zmm_t[:], 0.0)

        # strict-lower-triangular causal mask: tri[r, c] = 0 if c < r else
        # -1e30. Generated before gpsimd's DMA issues: it gates the first
        # exp via the diagonal-block mask add.
        tri_t = singles.tile([128, 128], f32)
        nc.gpsimd.memset(tri_t[:], 0.0)
        nc.gpsimd.affine_select(
            out=tri_t[:],
            in_=tri_t[:],
            compare_op=mybir.AluOpType.is_ge,
            fill=NEG_BIG,
            base=-1,
            pattern=[[-1, 128]],
            channel_multiplier=1,
        )

        # Process slots in ascending-span order: small slots ramp the
        # pipeline while the bigger transfers are still in flight.
        order = sorted(range(8), key=lambda s: spans[s])

        # DMA order = dependency order of the matmul->mask->exp chains.
        # Each engine's DGE runs one queue (issue ~0.7us, ~1.3us first-byte,
        # transfers serial per queue), so slots are spread over
        # sync/scalar/gpsimd by the time their data is consumed, and the
        # whole first chain (lhsT + a-row + first rhs) rides in ONE transfer.
        rhs_ts = [
            singles.tile([4, spans[s]], f32r, tag=f"rhs{s}", name=f"rhs{s}")
            for s in range(8)
        ]
        # first chain rides in a small head DMA: slot order[0]'s lhsT column
        # block sits at the head's front, its rhs (incl ones row) at the tail
        lhsT_t = singles.tile([4, 8 * 128 + spans[order[0]]], f32r)
        nc.sync.dma_start(
            lhsT_t[:, 8 * 128 :], lhsT_d[:, 8 * 128 :]
        )
        nc.sync.dma_start(
            lhsT_t[:, 128 * order[0] : 128 * (order[0] + 1)],
            lhsT_d[:, 128 * order[0] : 128 * (order[0] + 1)],
        )
        for s in (order[1], order[2]):
            nc.gpsimd.dma_start(
                rhs_ts[s][:], rhs_d[:, L * s : L * s + spans[s]]
            )
        for s in (order[6], order[7]):
            nc.scalar.dma_start(
                rhs_ts[s][:], rhs_d[:, L * s : L * s + spans[s]]
            )
        if order[0] > 0:
            nc.sync.dma_start(
                lhsT_t[:, 0 : 128 * order[0]], lhsT_d[:, 0 : 128 * order[0]]
            )
        nc.sync.dma_start(
            lhsT_t[:, 128 * (order[0] + 1) : 8 * 128],
            lhsT_d[:, 128 * (order[0] + 1) : 8 * 128],
        )
        for s in (order[3], order[4], order[5]):
            nc.sync.dma_start(
                rhs_ts[s][:], rhs_d[:, L * s : L * s + spans[s]]
            )

        # dummy exp on a zeroed element: forces the exp ACT_TABLE_LOAD
        # (~1.3us) into the idle preamble window instead of before the
        # first real ACTIVATE on the critical path
        warm_t = singles.tile([1, 1], f32)
        nc.gpsimd.memset(warm_t[:], 0.0)
        nc.scalar.activation(
            warm_t[:], warm_t[:], mybir.ActivationFunctionType.Exp
        )

        # one dummy fp32 matmul (~1.7us at 4 cyc/row) fills the PE-idle
        # DMA-latency window with activity, so the HAM clock gate flips to
        # 2.4 GHz shortly after the real matmuls start; sized to end right
        # as the first slot's data lands
        ps_warm = ps_pool.tile([128, pscols], f32, tag="ps")
        nc.tensor.matmul(
            ps_warm[:, 0:384],
            zmm_t[:, 0:128],
            zmm_t[:, 0:384],
            start=True,
            stop=True,
        )

        lam_t = singles.tile([128, 8], f32)

        for k, s in enumerate(order):
            cw = CW[s]
            rhs_ap = lhsT_t[:, 8 * 128 :] if s == order[0] else rhs_ts[s][:]
            ps = ps_pool.tile([128, pscols], f32, tag="ps")
            off = 0
            for g, w in enumerate(cw):
                nc.tensor.matmul(
                    ps[:, off : off + w],
                    lhsT_t[:, 128 * s : 128 * (s + 1)],
                    rhs_ap[:, off : off + w],
                    start=True,
                    stop=True,
                )
                off += w
            # causal mask on the diagonal block (last 128 cols of chunk 0)
            nc.vector.tensor_add(
                ps[:, cw[0] - 128 : cw[0]], ps[:, cw[0] - 128 : cw[0]], tri_t[:]
            )
            et = sc_pool.tile([128, pscols], f32)
            nc.scalar.activation(
                et[:, : spans[s]],
                ps[:, : spans[s]],
                mybir.ActivationFunctionType.Exp,
                accum_out=lam_t[:, s : s + 1],
            )

        nc.sync.dma_start(out_d[:], lam_t[:])

    nc.compile()
    return nc


def _pack_inputs(X, mu, alpha, beta, sigma):
    """Host-side f64 prep: per-core input dicts for the SPMD kernel.

    Returns (in_maps, mug_slots, W) where mug_slots[c] is the [128, 8] matrix
    of mu[cls] for the host-side finalize and W[s] is the chunk count of slot
    s (identical across cores; data-driven via the temporal-decay cutoff)."""
    t = X[..., 0].astype(np.float64)
    cls = X[..., 1].astype(np.int32)
    lon = X[..., 2].astype(np.float64)
    lat = X[..., 3].astype(np.float64)
    alpha = float(alpha)
    beta = float(beta)
    sigma = float(sigma)

    sig2 = sigma * sigma
    two_sig2 = 2.0 * sig2
    logc = math.log(alpha * beta / (math.pi * two_sig2))

    # per-batch centering (E is invariant; keeps fp32 magnitudes small)
    xc = lon - lon.mean(axis=1, keepdims=True)
    yc = lat - lat.mean(axis=1, keepdims=True)
    tc_ = t - t.mean(axis=1, keepdims=True)

    q = (xc * xc + yc * yc) / two_sig2
    a = logc - beta * tc_ - q          # [B, L]
    bv = beta * tc_ - q                # [B, L]
    rx = xc / sig2
    ry = yc / sig2
    mug = np.asarray(mu, np.float64)[cls]  # [B, L]

    # complementary row-tile pairs (i, 15-i), grouped so every core's slot s
    # sees nearly the same i (minimizes the max-over-cores span per slot):
    # group k holds i in {2k, 2k+1} -> 8 pairs, one per core.
    core_slots = []
    for c in range(NCORES):
        slots = []
        for k in range(4):
            b, i = c // 2, 2 * k + (c % 2)
            slots += [(b, i), (b, NRT - 1 - i)]
        core_slots.append(slots)

    # Temporal-decay cutoff: a dropped column contributes at most
    # exp(logc - beta*dt) to lam, and lam >= min(mu), so requiring
    # sum(dropped)/lam <= L*exp(logc - beta*dt_cut)/min(mu) <= e^-20 (~2e-9,
    # far below the reference's own f32 ulp) bounds the error of dropping
    # all history older than dt_cut. Falls back to the f32-underflow cut if
    # mu is not positive. Spans must be identical across cores (one SPMD
    # program) -> max over cores per slot, rounded to 128. Chunk 0 (ending
    # at the diagonal) is at most 512 wide; the rest splits into <=512
    # chunks.
    mu_min = float(np.min(np.asarray(mu, np.float64)))
    if beta > 0:
        if mu_min > 0:
            cut = (logc + math.log(L) - math.log(mu_min) + 20.0) / beta
        else:
            cut = (logc + 95.0) / beta
    else:
        cut = np.inf
    spans = [128] * 8
    for c in range(NCORES):
        for s, (b, i) in enumerate(core_slots[c]):
            d = 128 * (i + 1)
            j_min = int(np.searchsorted(t[b], t[b, 128 * i] - cut))
            need = -(-max(d - j_min, 128) // 128) * 128
            spans[s] = max(spans[s], min(need, d))
    CW = []
    for sp in spans:
        cw = [min(CHUNK, sp)]
        rem = sp - cw[0]
        while rem > 0:
            cw.append(min(CHUNK, rem))
            rem -= cw[-1]
        CW.append(cw)

    in_maps = []
    mug_slots = []
    for c in range(NCORES):
        slots = core_slots[c]
        # lhsT rows: [xc, yc, 1, a]; rhs rows: [rx, ry, bv, 1] ->
        # E = xc*rx + yc*ry + bv + a  (bias folded into the matmul).
        # The first-processed slot's rhs rides in lhsT's tail so the first
        # chain is one DMA.
        o0 = min(range(8), key=lambda s: (spans[s], s))
        lhsT = np.zeros((4, 8 * 128 + spans[o0]), np.float32)
        rhs = np.zeros((4, 8 * L), np.float32)
        mugp = np.zeros((128, 8), np.float64)
        for s, (b, i) in enumerate(slots):
            sp = spans[s]
            cw = CW[s]
            rows = slice(128 * i, 128 * (i + 1))
            lhsT[0, 128 * s : 128 * (s + 1)] = xc[b, rows]
            lhsT[1, 128 * s : 128 * (s + 1)] = yc[b, rows]
            lhsT[2, 128 * s : 128 * (s + 1)] = 1.0
            lhsT[3, 128 * s : 128 * (s + 1)] = a[b, rows]
            mugp[:, s] = mug[b, rows]

            # history span [d - sp, d) packed so chunk 0 ends exactly at the
            # diagonal, then earlier cols in <=512 chunks; padding cols (< 0)
            # are poisoned with b = -1e30 -> exp -> 0.
            d = 128 * (i + 1)
            lo = d - sp
            pad = -lo if lo < 0 else 0
            span = np.zeros((4, sp), np.float32)
            span[2, :pad] = NEG_BIG
            span[3, :] = 1.0
            cols = slice(max(lo, 0), d)
            span[0, pad:] = rx[b, cols]
            span[1, pad:] = ry[b, cols]
            span[2, pad:] = bv[b, cols]
            off = 0
            hi = sp
            for w in cw:
                rhs[:, L * s + off : L * s + off + w] = span[:, hi - w : hi]
                off += w
                hi -= w
        lhsT[:, 8 * 128 :] = rhs[:, L * o0 : L * o0 + spans[o0]]
        in_maps.append({"lhsT": lhsT, "rhs": rhs})
        mug_slots.append(mugp)
    return in_maps, mug_slots, CW


def kernel(X, mu, alpha, beta, sigma):
    global LAST_EXEC_NS
    X = np.asarray(X)
    mu64 = np.asarray(mu, np.float64)
    in_maps, mug_slots, W = _pack_inputs(X, mu, alpha, beta, sigma)
    nc = _build_nc(W)

    kwargs = {}
    if _PROFILE:
        kwargs = dict(trace=True, trace_cores=list(range(NCORES)), **_TRACE_KW)
    res = run_bass_kernel_spmd(nc, in_maps, core_ids=list(range(NCORES)), **kwargs)
    LAST_EXEC_NS = res.exec_time_ns

    sumlog = 0.0
    for c in range(NCORES):
        lam = res.results[c]["lam"].astype(np.float64)
        sumlog += float(np.log(lam + mug_slots[c] + EPS).sum())
    area = ((-0.30 - -0.42) * KM_PER_LON) * ((39.52 - 39.40) * KM_PER_LAT)
    baserate = float(mu64.sum()) * (T1 - T0) * area * B
    return np.float32(sumlog - baserate)
